# revision 26
# baseline (speedup 1.0000x reference)
"""Trainium2 Bass kernel for batched windowed multi-head attention.

Shapes: x (8, 64, 256, 512) f32, H=8 heads, D=64.
Sharding: data-parallel over batch dim B=8 -> 1 batch row per NeuronCore.

v3 design:
- x transposed on the HOST -> xT arrives via DMA (no PE transposes).
- exp(mask + pos_bias) precomputed on the HOST per (window, head), DMA'd
  bf16 ("emp"); softmax p = exp(scores) * emp.
- All matmuls bf16 (fp32 PSUM accumulation).
- Windows processed in PAIRS: weight-stationary projections (q/k/out)
  stream both windows' activations as one N=512 moving operand, halving
  matmul + evacuation op counts.
- Scores transposed (j on partitions), heads 2k/2k+1 row-packed (d=64
  contraction -> PE rows 0-63 / 64-127 run concurrently).
- attn@v col-packed per head pair; a ones[128,64] stationary produces
  PRE-BROADCAST softmax denominators in the same PSUM bank.
- v bias folded into the output-projection bias on the host
  (bp_eff = bp + Wp @ bv; softmax rows sum to 1).
- q/k evacuations ride on ScalarE (per-partition bias); out evac on
  VectorE scalar_tensor_tensor; emp-multiplies split GpSimd/VectorE.
"""
import os
import numpy as np
import ml_dtypes

import concourse.bass as bass
import concourse.mybir as mybir
import concourse.tile as tile
from concourse import bacc
from concourse.bass_utils import run_bass_kernel_spmd

B, W, S, E = 8, 64, 256, 512
H, D = 8, 64
SCALE = D ** -0.5
NCORES = 8
F32 = mybir.dt.float32
BF16 = mybir.dt.bfloat16
F8 = mybir.dt.float8e4
NPBF16 = ml_dtypes.bfloat16
NPF8 = ml_dtypes.float8_e4m3
AOp = mybir.AluOpType
AF = mybir.ActivationFunctionType
DR = mybir.MatmulPerfMode.DoubleRow


def _emit(nc, tc, ctx, n_g, d):
    """Emit the per-core program: n_g groups of 2 windows of MHA."""
    const = ctx.enter_context(tc.tile_pool(name="const", bufs=1))

    # --- one-time constants ---
    w_sb = {}
    for name in ("wq", "wk", "wv", "wp"):
        t = const.tile([128, 4, E], BF16, tag=name)
        nc.sync.dma_start(t[:], d[name][:])
        w_sb[name] = t
    bq_col = const.tile([128, 4], F32)
    nc.sync.dma_start(bq_col[:], d["bq"][:])
    bk_col = const.tile([128, 4], F32)
    nc.sync.dma_start(bk_col[:], d["bk"][:])
    bp_bc = const.tile([128, 4, 2, S], F32)
    nc.sync.dma_start(bp_bc[:], d["bp"][:])
    ones_den = const.tile([128, 64], BF16)
    nc.gpsimd.memset(ones_den[:], 1.0)

    # --- pools ---
    xt_p = ctx.enter_context(tc.tile_pool(name="xt", bufs=3))
    emp_p = ctx.enter_context(tc.tile_pool(name="emp", bufs=3))
    qkv_p = ctx.enter_context(tc.tile_pool(name="qkv", bufs=2))
    pe_p = ctx.enter_context(tc.tile_pool(name="pe", bufs=6))
    pp_p = ctx.enter_context(tc.tile_pool(name="pp", bufs=6))
    rec_p = ctx.enter_context(tc.tile_pool(name="rec", bufs=6))
    zt_p = ctx.enter_context(tc.tile_pool(name="zt", bufs=2))
    outs_p = ctx.enter_context(tc.tile_pool(name="outs", bufs=3))

    ps_proj = ctx.enter_context(tc.tile_pool(name="ps_proj", bufs=2, space="PSUM"))
    ps_sc = ctx.enter_context(tc.tile_pool(name="ps_sc", bufs=2, space="PSUM"))
    ps_zd = ctx.enter_context(tc.tile_pool(name="ps_zd", bufs=2, space="PSUM"))

    def phase_a(g):
        """DMA + projections for window pair g; returns tiles + chunk closures."""
        xT = xt_p.tile([128, 4, 2, S], BF16, tag="xT", name=f"xT{g}")
        nc.sync.dma_start(xT[:], d["x"][g])
        emp_t = [None, None]
        for wi in range(2):
            emp_t[wi] = emp_p.tile([128, H, 2, S], BF16, tag=f"emp{wi}",
                                   name=f"emp{g}_{wi}")
            nc.sync.dma_start(emp_t[wi][:], d["emp"][2 * g + wi])

        qT = qkv_p.tile([128, 4, 2, S], BF16, tag="qT", name=f"qT{g}")
        kT = qkv_p.tile([128, 4, 2, S], BF16, tag="kT", name=f"kT{g}")
        vA = [qkv_p.tile([128, 2, H, D], BF16, tag=f"vA{wi}", name=f"vA{g}_{wi}")
              for wi in range(2)]

        def qk_chunk(wt, dstT, bias_col, oc):
            # both windows ride in one N=512 moving operand
            pp = ps_proj.tile([128, 2, S], F32, tag="pj", name=f"pp{g}_{wt}_{oc}")
            for ic in range(4):
                nc.tensor.matmul(pp[:], w_sb[wt][:, ic, oc * 128:(oc + 1) * 128],
                                 xT[:, ic], start=(ic == 0), stop=(ic == 3))
            nc.scalar.activation(dstT[:, oc], pp[:], AF.Identity,
                                 bias=bias_col[:, oc:oc + 1])

        def v_chunk(wi, sc):
            pv = ps_proj.tile([128, E], F32, tag="pj", name=f"pv{g}_{wi}_{sc}")
            for ic in range(4):
                nc.tensor.matmul(pv[:], xT[:, ic, wi, sc * 128:(sc + 1) * 128],
                                 w_sb["wv"][:, ic], start=(ic == 0), stop=(ic == 3))
            nc.scalar.copy(vA[wi][:, sc], pv[:].rearrange("p (h v) -> p h v", h=H))

        chunks = []
        for oc in range(4):
            chunks.append(lambda oc=oc: qk_chunk("wq", qT, bq_col, oc))
            chunks.append(lambda oc=oc: qk_chunk("wk", kT, bk_col, oc))
        for wi in range(2):
            for sc in range(2):
                chunks.append(lambda wi=wi, sc=sc: v_chunk(wi, sc))
        return (qT, kT, vA, emp_t), chunks

    def phase_b(g, qT, kT, vA, emp_t):
        """Attention closures + output-projection tail for window pair g."""
        zT = zt_p.tile([128, 4, 2, S], BF16, tag="zT", name=f"zT{g}")
        pair_state = {}

        def pair_front(wi, k):
            # transposed scores, heads 2k / 2k+1 row-packed
            scp = ps_sc.tile([128, 2, 2, S], F32, tag="sc", name=f"sc{g}_{wi}_{k}")
            for jc in range(2):
                for a in range(2):
                    prow = a * 64
                    nc.tensor.matmul(scp[:, a, jc],
                                     kT[prow:prow + 64, k, wi, jc * 128:(jc + 1) * 128],
                                     qT[prow:prow + 64, k, wi], start=True, stop=True)
            pexp = pe_p.tile([128, 2, 2, S], BF16, tag="pexp", name=f"pe{g}_{wi}_{k}")
            nc.scalar.activation(pexp[:], scp[:], AF.Exp)
            p_sb = pp_p.tile([128, 2, 2, S], BF16, tag="p", name=f"p{g}_{wi}_{k}")
            eng = nc.gpsimd if (wi * 4 + k) % 2 == 0 else nc.vector
            eng.tensor_tensor(p_sb[:], pexp[:], emp_t[wi][:, 2 * k:2 * k + 2],
                              AOp.mult)
            pair_state[(wi, k)] = p_sb

        def back_pair(pa, pb):
            # za (half 0) + pre-broadcast denominators (half 1) for TWO
            # head-pairs, matmuls interleaved across their two PSUM banks
            # with alternating col-groups: every adjacent matmul overlaps in
            # the PE array, while each bank's accumulation groups stay
            # strictly sequential (start=True clears has_written bank-wide).
            st = {}
            for tag, (wi, k) in (("X", pa), ("Y", pb)):
                st[tag] = (wi, k, pair_state.pop((wi, k)),
                           ps_zd.tile([128, 2, S], F32, tag="zd",
                                      name=f"zd{g}_{wi}_{k}"))

            def mm(tag, half, a, jc):
                wi, k, p_sb, zd = st[tag]
                lhsT = vA[wi][:, jc, 2 * k + a] if half == 0 else ones_den[:]
                nc.tensor.matmul(zd[a * 64:(a + 1) * 64, half], lhsT,
                                 p_sb[:, a, jc], start=(jc == 0), stop=(jc == 1))

            for half in range(2):
                for gi in range(2):
                    for jc in range(2):
                        mm("X", half, gi, jc)
                        mm("Y", half, 1 - gi, jc)

            for tag in ("X", "Y"):
                wi, k, p_sb, zd = st[tag]
                rec = rec_p.tile([128, S], F32, tag="rec", name=f"rec{g}_{wi}_{k}")
                nc.vector.reciprocal_approx_fast(rec[:], zd[:, 1])
                nc.vector.tensor_tensor(zT[:, k, wi], zd[:, 0], rec[:], AOp.mult)

        def tail():
            outs = outs_p.tile([128, 4, 2, S], BF16, tag="osb", name=f"osb{g}")
            for oc in range(4):
                po = ps_proj.tile([128, 2, S], F32, tag="pj", name=f"po{g}_{oc}")
                for ec in range(4):
                    nc.tensor.matmul(po[:], w_sb["wp"][:, ec, oc * 128:(oc + 1) * 128],
                                     zT[:, ec], start=(ec == 0), stop=(ec == 3))
                nc.vector.scalar_tensor_tensor(
                    outs[:, oc], po[:], 0.0, bp_bc[:, oc], AOp.bypass, AOp.add)
            nc.sync.dma_start(d["out"][g], outs[:])

        fronts = [lambda wi=wi, k=k: pair_front(wi, k)
                  for wi in range(2) for k in range(4)]
        # DVE-multiplied pairs (odd k) first; GpSimd-multiplied pairs (even
        # k, ~2.1us multiply) last so their p tiles have time to land.
        bpairs = [lambda wi=wi: back_pair((wi, 1), (wi, 3)) for wi in range(2)]
        bpairs += [lambda wi=wi: back_pair((wi, 0), (wi, 2)) for wi in range(2)]
        return fronts, bpairs, tail

    prev = None
    for g in range(n_g):
        cur, chunks = phase_a(g)
        if prev is not None:
            fronts, bpairs, tail = phase_b(g - 1, *prev)
            # Interleave: score matmuls early (feed exp/emp-mult pipeline),
            # projection chunks of group g fill the vector-engine latency.
            seq = []
            ci = 0
            for i in range(4):
                seq.extend([fronts[2 * i], fronts[2 * i + 1]])
                seq.extend(chunks[ci:ci + 2]); ci += 2
            seq.append(bpairs[0]); seq.extend(chunks[ci:ci + 2]); ci += 2
            seq.append(bpairs[1]); seq.extend(chunks[ci:ci + 2]); ci += 2
            seq.extend([bpairs[2], bpairs[3]])
            seq.extend(chunks[ci:])
            seq.append(tail)
            for fn in seq:
                fn()
        else:
            for fn in chunks:
                fn()
        prev = cur
    fronts, bpairs, tail = phase_b(n_g - 1, *prev)
    for fn in fronts:
        fn()
    for fn in bpairs:
        fn()
    tail()


def _build(n_g):
    nc = bacc.Bacc("TRN2", target_bir_lowering=False, debug=False)
    d = {
        "x": nc.dram_tensor("x", [n_g, 128, 4, 2, S], BF16, kind="ExternalInput"),
        "emp": nc.dram_tensor("emp", [2 * n_g, 128, H, 2, S], BF16,
                              kind="ExternalInput"),
        "wq": nc.dram_tensor("wq", [128, 4, E], BF16, kind="ExternalInput"),
        "wk": nc.dram_tensor("wk", [128, 4, E], BF16, kind="ExternalInput"),
        "wv": nc.dram_tensor("wv", [128, 4, E], BF16, kind="ExternalInput"),
        "wp": nc.dram_tensor("wp", [128, 4, E], BF16, kind="ExternalInput"),
        "bq": nc.dram_tensor("bq", [128, 4], F32, kind="ExternalInput"),
        "bk": nc.dram_tensor("bk", [128, 4], F32, kind="ExternalInput"),
        "bp": nc.dram_tensor("bp", [128, 4, 2, S], F32, kind="ExternalInput"),
        "out": nc.dram_tensor("out", [n_g, 128, 4, 2, S], BF16,
                              kind="ExternalOutput"),
    }
    from contextlib import ExitStack
    with tile.TileContext(nc) as tc, ExitStack() as ctx:
        _emit(nc, tc, ctx, n_g, d)
    nc.compile()
    return nc


_NC_CACHE = {}


def _get_nc(n_g):
    if n_g not in _NC_CACHE:
        _NC_CACHE[n_g] = _build(n_g)
    return _NC_CACHE[n_g]


def _host_prep(mask, Wq, bq, Wk, bk, Wv, bv, Wp, bp, pos_bias, n_w):
    """Shared (replicated) tensors, host-side layout prep."""
    f = np.float32

    def wlay(wmat, scale=1.0):
        npdt = NPBF16
        # [out,in] torch Linear weight -> [128(e%128), ic, o], e=ic*128+p
        wt = np.asarray(wmat, f).T * scale
        return np.ascontiguousarray(
            wt.reshape(4, 128, E).transpose(1, 0, 2)).astype(npdt)

    def bcol(bvec, scale=1.0):
        # [o] -> [128(o%128), oc] f32
        return np.ascontiguousarray(
            (np.asarray(bvec, f) * scale).reshape(4, 128).T)

    # v bias folded into the output bias: out += bv @ Wp.T  (softmax rows
    # sum to 1), so v needs no bias on-device.
    bp_eff = np.asarray(bp, f) + np.asarray(Wp, f) @ np.asarray(bv, f)
    bp_b = np.ascontiguousarray(np.broadcast_to(
        bcol(bp_eff)[:, :, None, None], (128, 4, 2, S)).astype(f))

    # emp = exp(mask^T + pos_bias^T), [w, 128(j%128), h, jc, i] bf16
    mT = np.asarray(mask, f)[0, :n_w, 0].transpose(0, 2, 1)       # [w, j, i]
    pT = np.asarray(pos_bias, f).transpose(0, 2, 1)               # [h, j, i]
    emp = np.exp(mT[:, None] + pT[None])                          # [w, h, j, i]
    emp = emp.reshape(n_w, H, 2, 128, S).transpose(0, 3, 1, 2, 4)
    emp = np.ascontiguousarray(emp).astype(NPBF16)

    return {
        "wq": wlay(Wq, scale=SCALE), "wk": wlay(Wk), "wv": wlay(Wv),
        "wp": wlay(Wp),
        "bq": bcol(bq, SCALE), "bk": bcol(bk), "bp": bp_b,
        "emp": emp,
    }


def _x_lay(xc, n_w):
    # x[core] [w, s, e] -> [g, 128(e%128), ic, wi, s] fp8e4, e=ic*128+p
    xt = np.asarray(xc, np.float32)[:n_w].transpose(0, 2, 1)      # [w, e, s]
    xt = xt.reshape(n_w // 2, 2, 4, 128, S).transpose(0, 3, 2, 1, 4)
    return np.ascontiguousarray(xt).astype(NPBF16)


def kernel(x, mask, Wq, bq, Wk, bk, Wv, bv, Wp, bp, pos_bias, _trace=False):
    n_w = int(os.environ.get("KERNEL_NW", W))
    assert n_w % 2 == 0, "window count must be even (processed in pairs)"
    n_cores = NCORES
    x = np.asarray(x, np.float32)
    shared = _host_prep(mask, Wq, bq, Wk, bk, Wv, bv, Wp, bp, pos_bias, n_w)

    in_maps = []
    for c in range(n_cores):
        m = dict(shared)
        m["x"] = _x_lay(x[c % B], n_w)
        in_maps.append(m)

    nc = _get_nc(n_w // 2)
    res = run_bass_kernel_spmd(nc, in_maps, list(range(n_cores)), trace=_trace,
                               tmpdir=(os.environ.get("KERNEL_TRACE_DIR") if _trace else None))
    # out [g, 128(o%128), oc, wi, s] bf16 -> [w, s, o] f32
    outs = []
    for c in range(B):
        o = np.asarray(res.results[c]["out"]).astype(np.float32)
        o = o.transpose(0, 3, 2, 1, 4).reshape(n_w, E, S).transpose(0, 2, 1)
        outs.append(np.ascontiguousarray(o))
    out = np.stack(outs, axis=0)
    if _trace:
        kernel._last_exec_time_ns = res.exec_time_ns
        kernel._last_results = res
    return out


# revision 29
# speedup vs baseline: 1.2967x; 1.2967x over previous
"""Trainium2 Bass kernel for batched windowed multi-head attention.

Shapes: x (8, 64, 256, 512) f32, H=8 heads, D=64.
Sharding: data-parallel over batch dim B=8 -> 1 batch row per NeuronCore.

v3 design:
- x transposed on the HOST -> xT arrives via DMA (no PE transposes).
- exp(mask + pos_bias) precomputed on the HOST per (window, head), DMA'd
  bf16 ("emp"); softmax p = exp(scores) * emp.
- All matmuls bf16 (fp32 PSUM accumulation).
- Windows processed in PAIRS: weight-stationary projections (q/k/out)
  stream both windows' activations as one N=512 moving operand, halving
  matmul + evacuation op counts.
- Scores transposed (j on partitions), heads 2k/2k+1 row-packed (d=64
  contraction -> PE rows 0-63 / 64-127 run concurrently).
- attn@v col-packed per head pair; a ones[128,64] stationary produces
  PRE-BROADCAST softmax denominators in the same PSUM bank.
- v bias folded into the output-projection bias on the host
  (bp_eff = bp + Wp @ bv; softmax rows sum to 1).
- q/k evacuations ride on ScalarE (per-partition bias); out evac on
  VectorE scalar_tensor_tensor; emp-multiplies split GpSimd/VectorE.
"""
import os
import numpy as np
import ml_dtypes

import concourse.bass as bass
import concourse.mybir as mybir
import concourse.tile as tile
from concourse import bacc
from concourse.bass_utils import run_bass_kernel_spmd

B, W, S, E = 8, 64, 256, 512
H, D = 8, 64
SCALE = D ** -0.5
NCORES = 8
F32 = mybir.dt.float32
BF16 = mybir.dt.bfloat16
F8 = mybir.dt.float8e4
NPBF16 = ml_dtypes.bfloat16
NPF8 = ml_dtypes.float8_e4m3
AOp = mybir.AluOpType
AF = mybir.ActivationFunctionType
DR = mybir.MatmulPerfMode.DoubleRow


def _emit(nc, tc, ctx, n_g, d):
    """Emit the per-core program: n_g groups of 2 windows of MHA."""
    const = ctx.enter_context(tc.tile_pool(name="const", bufs=1))

    # --- one-time constants ---
    w_sb = {}
    for name in ("wq", "wk", "wv", "wp"):
        t = const.tile([128, 4, E], BF16, tag=name)
        nc.sync.dma_start(t[:], d[name][:])
        w_sb[name] = t
    bq_col = const.tile([128, 4], F32)
    nc.sync.dma_start(bq_col[:], d["bq"][:])
    bk_col = const.tile([128, 4], F32)
    nc.sync.dma_start(bk_col[:], d["bk"][:])
    bp_bc = const.tile([128, 4, 2, S], F32)
    nc.sync.dma_start(bp_bc[:], d["bp"][:])
    ones_den = const.tile([128, 64], BF16)
    nc.gpsimd.memset(ones_den[:], 1.0)

    # --- pools ---
    xt_p = ctx.enter_context(tc.tile_pool(name="xt", bufs=3))
    emp_p = ctx.enter_context(tc.tile_pool(name="emp", bufs=3))
    qkv_p = ctx.enter_context(tc.tile_pool(name="qkv", bufs=2))
    pe_p = ctx.enter_context(tc.tile_pool(name="pe", bufs=6))
    pp_p = ctx.enter_context(tc.tile_pool(name="pp", bufs=6))
    rec_p = ctx.enter_context(tc.tile_pool(name="rec", bufs=6))
    zt_p = ctx.enter_context(tc.tile_pool(name="zt", bufs=2))
    outs_p = ctx.enter_context(tc.tile_pool(name="outs", bufs=3))

    ps_proj = ctx.enter_context(tc.tile_pool(name="ps_proj", bufs=2, space="PSUM"))
    ps_sc = ctx.enter_context(tc.tile_pool(name="ps_sc", bufs=2, space="PSUM"))
    ps_zd = ctx.enter_context(tc.tile_pool(name="ps_zd", bufs=2, space="PSUM"))

    def phase_a(g):
        """DMA + projections for window pair g; returns tiles + chunk closures."""
        xT = xt_p.tile([128, 4, 2, S], BF16, tag="xT", name=f"xT{g}")
        nc.sync.dma_start(xT[:], d["x"][g])
        emp_t = [None, None]
        for wi in range(2):
            emp_t[wi] = emp_p.tile([128, H, 2, S], BF16, tag=f"emp{wi}",
                                   name=f"emp{g}_{wi}")
            nc.sync.dma_start(emp_t[wi][:], d["emp"][2 * g + wi])

        qT = qkv_p.tile([128, 4, 2, S], BF16, tag="qT", name=f"qT{g}")
        kT = qkv_p.tile([128, 4, 2, S], BF16, tag="kT", name=f"kT{g}")
        vA = [qkv_p.tile([128, 2, H, D], BF16, tag=f"vA{wi}", name=f"vA{g}_{wi}")
              for wi in range(2)]

        def qk_chunk(wt, dstT, bias_col, oc):
            # both windows ride in one N=512 moving operand
            pp = ps_proj.tile([128, 2, S], F32, tag="pj", name=f"pp{g}_{wt}_{oc}")
            for ic in range(4):
                nc.tensor.matmul(pp[:], w_sb[wt][:, ic, oc * 128:(oc + 1) * 128],
                                 xT[:, ic], start=(ic == 0), stop=(ic == 3))
            nc.scalar.activation(dstT[:, oc], pp[:], AF.Identity,
                                 bias=bias_col[:, oc:oc + 1])

        def v_chunk(wi, sc):
            pv = ps_proj.tile([128, E], F32, tag="pj", name=f"pv{g}_{wi}_{sc}")
            for ic in range(4):
                nc.tensor.matmul(pv[:], xT[:, ic, wi, sc * 128:(sc + 1) * 128],
                                 w_sb["wv"][:, ic], start=(ic == 0), stop=(ic == 3))
            nc.scalar.copy(vA[wi][:, sc], pv[:].rearrange("p (h v) -> p h v", h=H))

        chunks = []
        for oc in range(4):
            chunks.append(lambda oc=oc: qk_chunk("wq", qT, bq_col, oc))
            chunks.append(lambda oc=oc: qk_chunk("wk", kT, bk_col, oc))
        for wi in range(2):
            for sc in range(2):
                chunks.append(lambda wi=wi, sc=sc: v_chunk(wi, sc))
        return (qT, kT, vA, emp_t), chunks

    def phase_b(g, qT, kT, vA, emp_t):
        """Attention closures + output-projection tail for window pair g."""
        zT = zt_p.tile([128, 4, 2, S], BF16, tag="zT", name=f"zT{g}")
        pair_state = {}

        def pair_front(wi, k):
            # transposed scores, heads 2k / 2k+1 row-packed
            scp = ps_sc.tile([128, 2, 2, S], F32, tag="sc", name=f"sc{g}_{wi}_{k}")
            for jc in range(2):
                for a in range(2):
                    prow = a * 64
                    nc.tensor.matmul(scp[:, a, jc],
                                     kT[prow:prow + 64, k, wi, jc * 128:(jc + 1) * 128],
                                     qT[prow:prow + 64, k, wi], start=True, stop=True)
            pexp = pe_p.tile([128, 2, 2, S], BF16, tag="pexp", name=f"pe{g}_{wi}_{k}")
            nc.scalar.activation(pexp[:], scp[:], AF.Exp)
            p_sb = pp_p.tile([128, 2, 2, S], BF16, tag="p", name=f"p{g}_{wi}_{k}")
            eng = nc.gpsimd if (wi * 4 + k) % 2 == 0 else nc.vector
            eng.tensor_tensor(p_sb[:], pexp[:], emp_t[wi][:, 2 * k:2 * k + 2],
                              AOp.mult)
            pair_state[(wi, k)] = p_sb

        def pair_back(wi, k):
            p_sb = pair_state.pop((wi, k))
            # za (half 0) + pre-broadcast denominators (half 1); each
            # accumulation group runs to completion before the next group's
            # start=True (it clears has_written for the whole bank); groups on
            # alternating col-halves still overlap in the PE.
            zd = ps_zd.tile([128, 2, S], F32, tag="zd", name=f"zd{g}_{wi}_{k}")
            for a in range(2):
                for jc in range(2):
                    nc.tensor.matmul(zd[a * 64:(a + 1) * 64, 0],
                                     vA[wi][:, jc, 2 * k + a], p_sb[:, a, jc],
                                     start=(jc == 0), stop=(jc == 1))
            for a in range(2):
                for jc in range(2):
                    nc.tensor.matmul(zd[a * 64:(a + 1) * 64, 1],
                                     ones_den[:], p_sb[:, a, jc],
                                     start=(jc == 0), stop=(jc == 1))
            rec = rec_p.tile([128, S], F32, tag="rec", name=f"rec{g}_{wi}_{k}")
            nc.vector.reciprocal_approx_fast(rec[:], zd[:, 1])
            nc.vector.tensor_tensor(zT[:, k, wi], zd[:, 0], rec[:], AOp.mult)

        def tail():
            outs = outs_p.tile([128, 4, 2, S], BF16, tag="osb", name=f"osb{g}")
            for oc in range(4):
                po = ps_proj.tile([128, 2, S], F32, tag="pj", name=f"po{g}_{oc}")
                for ec in range(4):
                    nc.tensor.matmul(po[:], w_sb["wp"][:, ec, oc * 128:(oc + 1) * 128],
                                     zT[:, ec], start=(ec == 0), stop=(ec == 3))
                nc.vector.scalar_tensor_tensor(
                    outs[:, oc], po[:], 0.0, bp_bc[:, oc], AOp.bypass, AOp.add)
            nc.sync.dma_start(d["out"][g], outs[:])

        fronts = [lambda wi=wi, k=k: pair_front(wi, k)
                  for wi in range(2) for k in range(4)]
        backs = [lambda wi=wi, k=k: pair_back(wi, k)
                 for wi in range(2) for k in range(4)]
        return fronts, backs, tail

    prev = None
    for g in range(n_g):
        cur, chunks = phase_a(g)
        if prev is not None:
            fronts, backs, tail = phase_b(g - 1, *prev)
            # Interleave: score matmuls early (feed exp/emp-mult pipeline);
            # projection chunks of group g fill the vector-engine latency,
            # spread between the attn@v stages to cover the p-tile chain.
            seq = []
            ci = 0
            for i in range(4):
                seq.extend([fronts[2 * i], fronts[2 * i + 1]])
                seq.extend(chunks[ci:ci + 2]); ci += 2
            for i in range(4):
                seq.append(backs[i])
                seq.append(chunks[ci]); ci += 1
            seq.extend(backs[4:])
            seq.extend(chunks[ci:])
            seq.append(tail)
            for fn in seq:
                fn()
        else:
            for fn in chunks:
                fn()
        prev = cur
    fronts, backs, tail = phase_b(n_g - 1, *prev)
    for fn in fronts:
        fn()
    for fn in backs:
        fn()
    tail()


def _build(n_g):
    nc = bacc.Bacc("TRN2", target_bir_lowering=False, debug=False)
    d = {
        "x": nc.dram_tensor("x", [n_g, 128, 4, 2, S], BF16, kind="ExternalInput"),
        "emp": nc.dram_tensor("emp", [2 * n_g, 128, H, 2, S], BF16,
                              kind="ExternalInput"),
        "wq": nc.dram_tensor("wq", [128, 4, E], BF16, kind="ExternalInput"),
        "wk": nc.dram_tensor("wk", [128, 4, E], BF16, kind="ExternalInput"),
        "wv": nc.dram_tensor("wv", [128, 4, E], BF16, kind="ExternalInput"),
        "wp": nc.dram_tensor("wp", [128, 4, E], BF16, kind="ExternalInput"),
        "bq": nc.dram_tensor("bq", [128, 4], F32, kind="ExternalInput"),
        "bk": nc.dram_tensor("bk", [128, 4], F32, kind="ExternalInput"),
        "bp": nc.dram_tensor("bp", [128, 4, 2, S], F32, kind="ExternalInput"),
        "out": nc.dram_tensor("out", [n_g, 128, 4, 2, S], BF16,
                              kind="ExternalOutput"),
    }
    from contextlib import ExitStack
    with tile.TileContext(nc) as tc, ExitStack() as ctx:
        _emit(nc, tc, ctx, n_g, d)
    nc.compile()
    return nc


_NC_CACHE = {}


def _get_nc(n_g):
    if n_g not in _NC_CACHE:
        _NC_CACHE[n_g] = _build(n_g)
    return _NC_CACHE[n_g]


def _host_prep(mask, Wq, bq, Wk, bk, Wv, bv, Wp, bp, pos_bias, n_w):
    """Shared (replicated) tensors, host-side layout prep."""
    f = np.float32

    def wlay(wmat, scale=1.0):
        npdt = NPBF16
        # [out,in] torch Linear weight -> [128(e%128), ic, o], e=ic*128+p
        wt = np.asarray(wmat, f).T * scale
        return np.ascontiguousarray(
            wt.reshape(4, 128, E).transpose(1, 0, 2)).astype(npdt)

    def bcol(bvec, scale=1.0):
        # [o] -> [128(o%128), oc] f32
        return np.ascontiguousarray(
            (np.asarray(bvec, f) * scale).reshape(4, 128).T)

    # v bias folded into the output bias: out += bv @ Wp.T  (softmax rows
    # sum to 1), so v needs no bias on-device.
    bp_eff = np.asarray(bp, f) + np.asarray(Wp, f) @ np.asarray(bv, f)
    bp_b = np.ascontiguousarray(np.broadcast_to(
        bcol(bp_eff)[:, :, None, None], (128, 4, 2, S)).astype(f))

    # emp = exp(mask^T + pos_bias^T), [w, 128(j%128), h, jc, i] bf16
    mT = np.asarray(mask, f)[0, :n_w, 0].transpose(0, 2, 1)       # [w, j, i]
    pT = np.asarray(pos_bias, f).transpose(0, 2, 1)               # [h, j, i]
    emp = np.exp(mT[:, None] + pT[None])                          # [w, h, j, i]
    emp = emp.reshape(n_w, H, 2, 128, S).transpose(0, 3, 1, 2, 4)
    emp = np.ascontiguousarray(emp).astype(NPBF16)

    return {
        "wq": wlay(Wq, scale=SCALE), "wk": wlay(Wk), "wv": wlay(Wv),
        "wp": wlay(Wp),
        "bq": bcol(bq, SCALE), "bk": bcol(bk), "bp": bp_b,
        "emp": emp,
    }


def _x_lay(xc, n_w):
    # x[core] [w, s, e] -> [g, 128(e%128), ic, wi, s] fp8e4, e=ic*128+p
    xt = np.asarray(xc, np.float32)[:n_w].transpose(0, 2, 1)      # [w, e, s]
    xt = xt.reshape(n_w // 2, 2, 4, 128, S).transpose(0, 3, 2, 1, 4)
    return np.ascontiguousarray(xt).astype(NPBF16)


def kernel(x, mask, Wq, bq, Wk, bk, Wv, bv, Wp, bp, pos_bias, _trace=False):
    n_w = int(os.environ.get("KERNEL_NW", W))
    assert n_w % 2 == 0, "window count must be even (processed in pairs)"
    n_cores = NCORES
    x = np.asarray(x, np.float32)
    shared = _host_prep(mask, Wq, bq, Wk, bk, Wv, bv, Wp, bp, pos_bias, n_w)

    in_maps = []
    for c in range(n_cores):
        m = dict(shared)
        m["x"] = _x_lay(x[c % B], n_w)
        in_maps.append(m)

    nc = _get_nc(n_w // 2)
    res = run_bass_kernel_spmd(nc, in_maps, list(range(n_cores)), trace=_trace,
                               tmpdir=(os.environ.get("KERNEL_TRACE_DIR") if _trace else None))
    # out [g, 128(o%128), oc, wi, s] bf16 -> [w, s, o] f32
    outs = []
    for c in range(B):
        o = np.asarray(res.results[c]["out"]).astype(np.float32)
        o = o.transpose(0, 3, 2, 1, 4).reshape(n_w, E, S).transpose(0, 2, 1)
        outs.append(np.ascontiguousarray(o))
    out = np.stack(outs, axis=0)
    if _trace:
        kernel._last_exec_time_ns = res.exec_time_ns
        kernel._last_results = res
    return out
